# revision 1
# baseline (speedup 1.0000x reference)
"""GIN-style GNN message-passing layer on 8 Trainium2 NeuronCores.

Math (per reference):
    m      = h[src] + edge_attr                       [E, 96]
    aggr   = segment_sum(m, dst, N)                   [N, 96]
    out    = (1+eps)*h + relu(aggr @ W1 + b1) @ W2 + b2

Distribution strategy (node-parallel, zero collectives):
  Destination nodes are packed on the host into 400 "windows" of <=128 nodes
  such that each window's incident edges fit in a fixed number of 128-edge
  chunks; core k owns 50 windows. Every edge belongs to exactly one window
  (its dst), so aggregation is core-local. Per chunk the device:
    - gathers h[src] rows with the GPSIMD gather-DMA (int16 indices; the
      32767 index limit is handled by splitting each window's edges into
      src<25000 and src>=25000 streams, the second gathered through an
      offset view of the table),
    - builds a 128x128 one-hot dst indicator on DVE (iota == dst_rel),
    - scatter-adds via TensorE: PSUM[node,emb] += indicator.T @ msgs,
  accumulating h-part and edge_attr-part as two matmuls into one PSUM tile.
  The per-node MLP + GIN update then runs on the 128-node window and the
  result is DMA'd out. Host un-permutes the shards into the full output.
"""
import os
import numpy as np
import ml_dtypes

import concourse.bass as bass
import concourse.mybir as mybir
import concourse.tile as tile
from concourse import bacc
from concourse.bass_utils import run_bass_kernel_spmd
from concourse.masks import make_identity

# problem shape (hardcoded per contest contract)
N_NODES = 50000
N_EDGES = 800000
EMB = 96
HID = 192
P = 128
N_CORES = 8
W_PER_CORE = 50
# windows per gather call-pair. Keep gather calls at 1024 indices: larger
# calls (2048+) overflow runtime DMA state and crash NRT (HW-verified).
GRP = 1
N_WINDOWS = N_CORES * W_PER_CORE
SPLIT = 25000

# message/scatter stage dtype: bf16 halves gather+edge DMA traffic and
# speeds the indicator matmuls (FWL); MLP stays f32 either way.
MSG_BF16 = os.environ.get("GNN_MSG_BF16", "1") == "1"

LAST_RESULTS = None      # BassKernelResults of the most recent run (for test.py)
_PROGRAM_CACHE = {}


# ----------------------------------------------------------------- host plan
def _pack_windows(deg_lo, deg_hi, n_windows, cap_half, max_nodes=P):
    order = np.argsort(-(deg_lo + deg_hi), kind="stable")
    lo_left = np.full(n_windows, cap_half, dtype=np.int64)
    hi_left = np.full(n_windows, cap_half, dtype=np.int64)
    slots_left = np.full(n_windows, max_nodes, dtype=np.int64)
    win_of_node = np.full(len(deg_lo), -1, dtype=np.int64)
    ptr = 0
    for v in order:
        dl, dh = deg_lo[v], deg_hi[v]
        for off in range(n_windows):
            w = (ptr + off) % n_windows
            if slots_left[w] > 0 and lo_left[w] >= dl and hi_left[w] >= dh:
                win_of_node[v] = w
                slots_left[w] -= 1
                lo_left[w] -= dl
                hi_left[w] -= dh
                ptr = (w + 1) % n_windows
                break
        else:
            return None
    return win_of_node


def _build_plan(src, dst):
    src = np.asarray(src).astype(np.int64)
    dst = np.asarray(dst).astype(np.int64)
    is_hi = src >= SPLIT

    deg_lo = np.bincount(dst[~is_hi], minlength=N_NODES)
    deg_hi = np.bincount(dst[is_hi], minlength=N_NODES)

    c_half = None
    base = max(1, int(np.ceil(max(deg_lo.sum(), deg_hi.sum()) / N_WINDOWS / P)))
    for c in range(base, 40):
        win_of_node = _pack_windows(deg_lo, deg_hi, N_WINDOWS, c * P)
        if win_of_node is not None:
            c_half = c
            break
    assert c_half is not None, "window packing failed"

    # dense slot of each node inside its window
    order = np.argsort(win_of_node, kind="stable")
    starts = np.searchsorted(win_of_node[order], np.arange(N_WINDOWS))
    slot_sorted = np.arange(N_NODES) - starts[win_of_node[order]]
    slot_of_node = np.empty(N_NODES, dtype=np.int64)
    slot_of_node[order] = slot_sorted

    C = 2 * c_half
    s_win = C * P
    n_slots = N_WINDOWS * s_win

    ew = win_of_node[dst]
    ekey = ew * 2 + is_hi
    eorder = np.argsort(ekey, kind="stable")
    cnt = np.bincount(ekey, minlength=2 * N_WINDOWS)
    assert cnt.max() <= c_half * P

    block_base = np.zeros(2 * N_WINDOWS, dtype=np.int64)
    block_base[0::2] = np.arange(N_WINDOWS) * s_win
    block_base[1::2] = np.arange(N_WINDOWS) * s_win + c_half * P
    within = np.arange(N_EDGES) - np.repeat(
        np.concatenate([[0], np.cumsum(cnt)[:-1]]), cnt)
    edge_at_slot = np.full(n_slots, -1, dtype=np.int64)
    edge_at_slot[block_base[ekey[eorder]] + within] = eorder

    pad = edge_at_slot < 0
    e_safe = np.where(pad, 0, edge_at_slot)
    slot_src = np.where(pad, 0, src[e_safe])
    hi_chunk = (np.arange(n_slots) // P) % C >= c_half
    slot_gidx = np.where(hi_chunk, np.where(pad, 0, slot_src - SPLIT), slot_src)
    assert slot_gidx.min() >= 0 and slot_gidx.max() < 32768
    slot_dstrel = np.where(pad, -1.0,
                           slot_of_node[np.where(pad, 0, dst[e_safe])]).astype(np.float32)

    return dict(c_half=c_half, C=C, win_of_node=win_of_node,
                slot_of_node=slot_of_node, edge_at_slot=edge_at_slot,
                slot_gidx=slot_gidx, slot_dstrel=slot_dstrel, pad=pad)


def _wrap_idx_blocks(g):
    """[n_win, num] -> [n_win, 128, num//16] int16 (16-partition wrap, x8 replicate)."""
    n_win, num = g.shape
    t = g.reshape(n_win, num // 16, 16).transpose(0, 2, 1).astype(np.int16)
    return np.tile(t, (1, 8, 1))


# -------------------------------------------------------------- device build
def _build_program(c_half):
    C = 2 * c_half
    f32 = mybir.dt.float32
    mdt = mybir.dt.bfloat16 if MSG_BF16 else f32

    nc = bacc.Bacc("TRN2", target_bir_lowering=False, debug=False,
                   num_devices=N_CORES)
    t_htable = nc.dram_tensor("h_table", [N_NODES, 128], mdt, kind="ExternalInput")
    t_ea = nc.dram_tensor("ea", [W_PER_CORE, P, C * EMB], mdt, kind="ExternalInput")
    t_gidx = nc.dram_tensor("gidx", [W_PER_CORE // GRP, P, GRP * C * 8],
                            mybir.dt.int16, kind="ExternalInput")
    t_dstrel = nc.dram_tensor("dstrel", [W_PER_CORE, P, C], mdt, kind="ExternalInput")
    t_hres = nc.dram_tensor("hres", [W_PER_CORE * P, EMB], f32, kind="ExternalInput")
    t_w1 = nc.dram_tensor("w1", [EMB, HID], f32, kind="ExternalInput")
    t_b1 = nc.dram_tensor("b1", [HID, 1], f32, kind="ExternalInput")
    t_w2 = nc.dram_tensor("w2", [HID, EMB], f32, kind="ExternalInput")
    t_b2bc = nc.dram_tensor("b2bc", [P, EMB], f32, kind="ExternalInput")
    t_epsb = nc.dram_tensor("epsb", [P, 1], f32, kind="ExternalInput")
    t_out = nc.dram_tensor("out", [W_PER_CORE * P, EMB], f32, kind="ExternalOutput")

    with tile.TileContext(nc) as tc:
        with (
            tc.tile_pool(name="const", bufs=1) as cpool,
            tc.tile_pool(name="work", bufs=3) as wpool,
            tc.tile_pool(name="small", bufs=3) as spool,
            tc.tile_pool(name="psuma", bufs=2, space="PSUM") as ppool_a,
            tc.tile_pool(name="psumb", bufs=2, space="PSUM") as ppool_b,
            tc.tile_pool(name="psumc", bufs=1, space="PSUM") as ppool_c,
        ):
            ident = cpool.tile([P, P], f32)
            make_identity(nc, ident[:])
            iota_i = cpool.tile([P, C * P], mybir.dt.int32)
            nc.gpsimd.iota(iota_i[:].rearrange("p (c j) -> p c j", c=C),
                           [[0, C], [1, P]], base=0, channel_multiplier=0)
            iota_f = cpool.tile([P, C * P], mdt)
            nc.vector.tensor_copy(iota_f[:], iota_i[:])
            w1_t = cpool.tile([EMB, HID], f32)
            nc.sync.dma_start(out=w1_t[:], in_=t_w1[:])
            w2a_t = cpool.tile([EMB, EMB], f32)
            nc.sync.dma_start(out=w2a_t[:], in_=t_w2[0:EMB, :])
            w2b_t = cpool.tile([EMB, EMB], f32)
            nc.sync.dma_start(out=w2b_t[:], in_=t_w2[EMB:HID, :])
            b1a = cpool.tile([EMB, 1], f32)
            nc.sync.dma_start(out=b1a[:], in_=t_b1[0:EMB, :])
            b1b = cpool.tile([EMB, 1], f32)
            nc.sync.dma_start(out=b1b[:], in_=t_b1[EMB:HID, :])
            b2bc = cpool.tile([P, EMB], f32)
            nc.sync.dma_start(out=b2bc[:], in_=t_b2bc[:])
            scale = cpool.tile([P, 1], f32)
            nc.sync.dma_start(out=scale[:], in_=t_epsb[:])
            nc.vector.tensor_scalar_add(scale[:], scale[:], 1.0)

            gath = None
            for w in range(W_PER_CORE):
                g, wl = divmod(w, GRP)
                if wl == 0:
                    # one gather pair per GRP-window group: fewer SWDGE calls,
                    # less serialized Q7 descriptor-generation time
                    gath = wpool.tile([P, 2, GRP * c_half, 128], mdt, tag="gath")
                    gidx_t = spool.tile([P, GRP * C * 8], mybir.dt.int16, tag="gidx")
                    nc.sync.dma_start(out=gidx_t[:], in_=t_gidx[g])
                    nc.gpsimd.dma_gather(
                        out_ap=gath[:, 0], in_ap=t_htable[:],
                        idxs_ap=gidx_t[:, 0:GRP * c_half * 8],
                        num_idxs=GRP * c_half * P, num_idxs_reg=GRP * c_half * P,
                        elem_size=128)
                    nc.gpsimd.dma_gather(
                        out_ap=gath[:, 1], in_ap=t_htable[SPLIT:, :],
                        idxs_ap=gidx_t[:, GRP * c_half * 8:],
                        num_idxs=GRP * c_half * P, num_idxs_reg=GRP * c_half * P,
                        elem_size=128)

                ea_t = wpool.tile([P, C, EMB], mdt, tag="ea")
                nc.sync.dma_start(out=ea_t[:],
                                  in_=t_ea[w].rearrange("p (c e) -> p c e", c=C))
                dst_t = spool.tile([P, C], mdt, tag="dst")
                nc.sync.dma_start(out=dst_t[:], in_=t_dstrel[w])

                ind = wpool.tile([P, C, P], mdt, tag="ind")
                nc.vector.tensor_tensor(
                    out=ind[:], in0=iota_f[:].rearrange("p (c j) -> p c j", c=C),
                    in1=dst_t[:].to_broadcast([P, C, P]),
                    op=mybir.AluOpType.is_equal)

                aggr_p = ppool_a.tile([P, EMB], f32, tag="aggr")
                for c in range(C):
                    s, cs = (0, c) if c < c_half else (1, c - c_half)
                    nc.tensor.matmul(aggr_p[:], lhsT=ind[:, c, :],
                                     rhs=gath[:, s, wl * c_half + cs, 0:EMB],
                                     start=(c == 0), stop=False)
                    nc.tensor.matmul(aggr_p[:], lhsT=ind[:, c, :],
                                     rhs=ea_t[:, c, :],
                                     start=False, stop=(c == C - 1))

                aggr_s = spool.tile([P, EMB], f32, tag="aggr_s")
                nc.scalar.copy(aggr_s[:], aggr_p[:])
                aggrT_p = ppool_c.tile([EMB, P], f32, tag="aggrT")
                nc.tensor.transpose(aggrT_p[:], aggr_s[:], ident[:])
                aggrT_s = spool.tile([EMB, P], f32, tag="aggrT_s")
                nc.scalar.copy(aggrT_s[:], aggrT_p[:])

                h1_p = ppool_c.tile([EMB, P], f32, tag="h1")
                nc.tensor.matmul(h1_p[:], lhsT=w1_t[:, 0:EMB], rhs=aggrT_s[:],
                                 start=True, stop=True)
                h2_p = ppool_c.tile([EMB, P], f32, tag="h2")
                nc.tensor.matmul(h2_p[:], lhsT=w1_t[:, EMB:HID], rhs=aggrT_s[:],
                                 start=True, stop=True)
                h1_s = spool.tile([EMB, P], f32, tag="h1s")
                nc.scalar.activation(h1_s[:], h1_p[:],
                                     mybir.ActivationFunctionType.Relu, bias=b1a[:])
                h2_s = spool.tile([EMB, P], f32, tag="h2s")
                nc.scalar.activation(h2_s[:], h2_p[:],
                                     mybir.ActivationFunctionType.Relu, bias=b1b[:])

                out_p = ppool_b.tile([P, EMB], f32, tag="outp")
                nc.tensor.matmul(out_p[:], lhsT=h1_s[:], rhs=w2a_t[:],
                                 start=True, stop=False)
                nc.tensor.matmul(out_p[:], lhsT=h2_s[:], rhs=w2b_t[:],
                                 start=False, stop=True)

                hres_t = spool.tile([P, EMB], f32, tag="hres")
                nc.sync.dma_start(out=hres_t[:], in_=t_hres[w * P:(w + 1) * P, :])
                out_t = spool.tile([P, EMB], f32, tag="out")
                nc.vector.tensor_scalar(out_t[:], hres_t[:], scale[:, 0:1], None,
                                        op0=mybir.AluOpType.mult)
                nc.vector.tensor_tensor(out_t[:], out_t[:], out_p[:],
                                        op=mybir.AluOpType.add)
                nc.vector.tensor_tensor(out_t[:], out_t[:], b2bc[:],
                                        op=mybir.AluOpType.add)
                nc.sync.dma_start(out=t_out[w * P:(w + 1) * P, :], in_=out_t[:])

    nc.compile()
    return nc


# ------------------------------------------------------------------- kernel
def kernel(h, edge_attr, src, dst, W1, b1, W2, b2, eps):
    global LAST_RESULTS
    h = np.asarray(h, dtype=np.float32)
    edge_attr = np.asarray(edge_attr, dtype=np.float32)
    W1 = np.asarray(W1, dtype=np.float32)
    b1 = np.asarray(b1, dtype=np.float32)
    W2 = np.asarray(W2, dtype=np.float32)
    b2 = np.asarray(b2, dtype=np.float32)
    eps = np.asarray(eps, dtype=np.float32)

    plan = _build_plan(src, dst)
    c_half, C = plan["c_half"], plan["C"]
    s_win = C * P
    mnp = ml_dtypes.bfloat16 if MSG_BF16 else np.float32

    if c_half not in _PROGRAM_CACHE:
        _PROGRAM_CACHE[c_half] = _build_program(c_half)
    nc = _PROGRAM_CACHE[c_half]

    # ---- per-slot host arrays (global, then sliced per core) ----
    ea_slots = np.zeros((N_WINDOWS * s_win, EMB), dtype=mnp)
    valid = ~plan["pad"]
    ea_slots[valid] = edge_attr[plan["edge_at_slot"][valid]].astype(mnp)
    # [n_win, C, P, EMB] -> p-major [n_win, P, C*EMB]
    ea_pm = np.ascontiguousarray(
        ea_slots.reshape(N_WINDOWS, C, P, EMB).transpose(0, 2, 1, 3)
    ).reshape(N_WINDOWS, P, C * EMB)

    dstrel_pm = np.ascontiguousarray(
        plan["slot_dstrel"].reshape(N_WINDOWS, C, P).transpose(0, 2, 1)
    ).astype(mnp)

    # group gather indices: [n_groups, stream, GRP windows * c_half chunks * 128]
    n_groups = N_WINDOWS // GRP
    G = plan["slot_gidx"].reshape(n_groups, GRP, C, P)
    gidx_in = np.concatenate([
        _wrap_idx_blocks(G[:, :, :c_half].reshape(n_groups, GRP * c_half * P)),
        _wrap_idx_blocks(G[:, :, c_half:].reshape(n_groups, GRP * c_half * P)),
    ], axis=2)

    hres = np.zeros((N_WINDOWS * P, EMB), dtype=np.float32)
    hres[plan["win_of_node"] * P + plan["slot_of_node"]] = h

    h_table = np.zeros((N_NODES, 128), dtype=mnp)
    h_table[:, :EMB] = h.astype(mnp)

    b2bc = np.tile(b2[None, :], (P, 1)).astype(np.float32)
    epsb = np.full((P, 1), eps[0], dtype=np.float32)

    in_maps = []
    gpc = W_PER_CORE // GRP          # gather groups per core
    for k in range(N_CORES):
        ws = slice(k * W_PER_CORE, (k + 1) * W_PER_CORE)
        gs = slice(k * gpc, (k + 1) * gpc)
        rs = slice(k * W_PER_CORE * P, (k + 1) * W_PER_CORE * P)
        in_maps.append(dict(
            h_table=h_table, ea=ea_pm[ws], gidx=gidx_in[gs],
            dstrel=dstrel_pm[ws], hres=hres[rs],
            w1=W1, b1=b1[:, None], w2=W2, b2bc=b2bc, epsb=epsb))

    LAST_RESULTS = run_bass_kernel_spmd(nc, in_maps, core_ids=list(range(N_CORES)),
                                        tmpdir=os.environ.get("GNN_TRACE_DIR") or None)
    shards = np.concatenate([LAST_RESULTS.results[k]["out"]
                             for k in range(N_CORES)], axis=0)
    out = shards[plan["win_of_node"] * P + plan["slot_of_node"]]
    return np.ascontiguousarray(out, dtype=np.float32)

